# revision 28
# baseline (speedup 1.0000x reference)
"""Multi-head attention (B=2, F=T=2048, H=1024, 16 heads x 64) on 8 TRN2
NeuronCores.

Sharding (v2): pure head/tensor parallelism with an output-side AllToAll.
Core c owns heads {2c, 2c+1} for BOTH batches. Each core:
  1. projects Q^T / K^T / V for its 2 heads over the full sequences,
  2. runs attention for its heads (softmax denominators come free from a
     ones-column appended to V in the P@V matmul; exp runs on the scalar
     engine with the 1/sqrt(64) logit scale folded into its free affine),
  3. normalizes A^T off the PE (reciprocal_approx_fast on DVE + gpsimd
     partition_broadcast), then
  4. one 8-core AllToAll redistributes A^T from head-sharded to
     (batch, query-slice)-sharded, and the output projection runs locally
     with the full 1024-deep head contraction -> exact [512, 1024] slice.
Host concatenates the 8 slices. All matmuls run in bf16 with fp32 PSUM
accumulation.
"""

from contextlib import ExitStack

import ml_dtypes
import numpy as np

import concourse.bass as bass  # noqa: F401
import concourse.mybir as mybir
import concourse.tile as tile
from concourse import bacc
from concourse.bass_utils import run_bass_kernel_spmd

B, F, T, HID, NH, DH = 2, 2048, 2048, 1024, 16, 64
FS = F // 4  # 512-row output slice per core
HT = HID // 128  # 8 h-tiles
TT = T // 128  # 16 key tiles
FC = F // 512  # 4 query chunks
BF16, F32 = mybir.dt.bfloat16, mybir.dt.float32
NPBF16 = ml_dtypes.bfloat16

_CACHE: dict = {}


def _build():
    nc = bacc.Bacc("TRN2", target_bir_lowering=False, debug=False, num_devices=8)

    qT = nc.declare_dram_parameter("qT", [B, HID, F], BF16, isOutput=False)
    sT = nc.declare_dram_parameter("sT", [B, HID, T], BF16, isOutput=False)
    wq = nc.declare_dram_parameter("wq", [HID, 128], BF16, isOutput=False)
    wk = nc.declare_dram_parameter("wk", [HID, 128], BF16, isOutput=False)
    wv = nc.declare_dram_parameter("wv", [HID, 128], BF16, isOutput=False)
    wo = nc.declare_dram_parameter("wo", [HID, HID], BF16, isOutput=False)
    out = nc.declare_dram_parameter("out", [FS, HID], F32, isOutput=True)

    seg = 128 * FS  # one A^T shard: [128 hd, 512 f]
    a2a_in = nc.dram_tensor("a2a_in", [8, seg], BF16)
    a2a_out = nc.dram_tensor("a2a_out", [8, seg], BF16)

    with tile.TileContext(nc) as tc, ExitStack() as ctx:
        persist = ctx.enter_context(tc.tile_pool(name="persist", bufs=1))
        kT_sb = persist.tile([128, B, T], BF16, tag="kT")
        v_sb = persist.tile([128, B, TT, 2, DH + 1], BF16, tag="v")
        qTp_sb = persist.tile([128, B, F], BF16, tag="qTp")
        wo_sb = persist.tile([128, HT, HID], BF16, tag="wo")
        w3_sb = persist.tile([128, HT, 3, 128], BF16, tag="w3")  # wq|wk|wv
        ones_sb = persist.tile([128, DH, 1], BF16, tag="ones")

        nc.vector.memset(ones_sb[:, :, :], 1.0)
        nc.vector.memset(v_sb[:, :, :, :, DH : DH + 1], 1.0)
        nc.sync.dma_start(
            out=w3_sb[:, :, 0, :], in_=wq[:, :].rearrange("(a p) n -> p a n", p=128)
        )
        nc.sync.dma_start(
            out=w3_sb[:, :, 1, :], in_=wk[:, :].rearrange("(a p) n -> p a n", p=128)
        )
        nc.sync.dma_start(
            out=w3_sb[:, :, 2, :], in_=wv[:, :].rearrange("(a p) n -> p a n", p=128)
        )

        with (
            tc.tile_pool(name="inp", bufs=1) as inp_pool,
            tc.tile_pool(name="inps", bufs=2) as inps_pool,
            tc.tile_pool(name="ptp", bufs=4) as pt_pool,
            tc.tile_pool(name="rtp", bufs=2) as rt_pool,
            tc.tile_pool(name="stg", bufs=2) as stg_pool,
        ):
            # ---- projections (both batches, PSUM scope closes after) --
            with tc.tile_pool(name="proj_ps", bufs=2, space="PSUM") as proj_ps:
                qT_tiles = []
                for b in range(B):
                    sT_sb = inps_pool.tile([128, HT, T], BF16, tag="sT")
                    nc.sync.dma_start(
                        out=sT_sb[:, :, :],
                        in_=sT[b, :, :].rearrange("(a p) n -> p a n", p=128),
                    )
                    # K^T [128 hd, T]
                    for c in range(T // 512):
                        ps = proj_ps.tile([128, 512], F32, tag="ps")
                        for ht in range(HT):
                            nc.tensor.matmul(
                                ps[:, :],
                                lhsT=w3_sb[:, ht, 1, :],
                                rhs=sT_sb[:, ht, 512 * c : 512 * (c + 1)],
                                start=(ht == 0),
                                stop=(ht == HT - 1),
                            )
                        nc.vector.tensor_copy(
                            out=kT_sb[:, b, 512 * c : 512 * (c + 1)], in_=ps[:, :]
                        )
                    # V [t, 2*DH] per key tile
                    for tt in range(TT):
                        ps = proj_ps.tile([128, 128], F32, tag="ps")
                        for ht in range(HT):
                            nc.tensor.matmul(
                                ps[:, :],
                                lhsT=sT_sb[:, ht, 128 * tt : 128 * (tt + 1)],
                                rhs=w3_sb[:, ht, 2, :],
                                start=(ht == 0),
                                stop=(ht == HT - 1),
                            )
                        nc.vector.tensor_copy(
                            out=v_sb[:, b, tt, :, 0:DH],
                            in_=ps[:, :].rearrange("p (j d) -> p j d", j=2),
                        )
                    # Q^T [128 hd, F]
                    qT_sb = inp_pool.tile([128, HT, F], BF16, tag="qT")
                    nc.scalar.dma_start(
                        out=qT_sb[:, :, :],
                        in_=qT[b, :, :].rearrange("(a p) n -> p a n", p=128),
                    )
                    for c in range(FC):
                        ps = proj_ps.tile([128, 512], F32, tag="ps")
                        for ht in range(HT):
                            nc.tensor.matmul(
                                ps[:, :],
                                lhsT=w3_sb[:, ht, 0, :],
                                rhs=qT_sb[:, ht, 512 * c : 512 * (c + 1)],
                                start=(ht == 0),
                                stop=(ht == HT - 1),
                            )
                        nc.vector.tensor_copy(
                            out=qTp_sb[:, b, 512 * c : 512 * (c + 1)], in_=ps[:, :]
                        )

            # ---- attention (both batches); scale chains deferred by one
            # fc so the in-order PE never stalls on recip/broadcast ------
            with (
                tc.tile_pool(name="s_ps", bufs=2, space="PSUM") as s_ps_pool,
                tc.tile_pool(name="a_ps", bufs=2, space="PSUM") as a_ps_pool,
            ):

                def flush_scale(b, fc, a_ps):
                    shard = a2a_in[4 * b + fc, :].rearrange(
                        "(p n) -> p n", p=128
                    )
                    rts = []
                    for j in range(2):
                        rt = rt_pool.tile([65, 1, 512], BF16, tag="rt")
                        with nc.allow_low_precision("bf16 softmax denom recip"):
                            nc.vector.reciprocal(
                                out=rt[64:65, 0, :], in_=a_ps[64:65, j, :]
                            )
                        rts.append(rt)
                    for j in range(2):
                        rt = rts[j]
                        bc = s_ps_pool.tile([64, 512], F32, tag="s")
                        nc.tensor.matmul(
                            bc[:, :],
                            lhsT=ones_sb[64:65, :, 0],
                            rhs=rt[64:65, 0, :],
                            start=True,
                            stop=True,
                        )
                        bc_sb = rt_pool.tile([64, 512], F32, tag="bc")
                        nc.vector.tensor_copy(out=bc_sb[:, :], in_=bc[:, :])
                        st = stg_pool.tile([64, 512], BF16, tag="st")
                        nc.vector.tensor_mul(
                            out=st[:, :], in0=a_ps[0:64, j, :], in1=bc_sb[:, :]
                        )
                        nc.sync.dma_start(
                            out=shard[64 * j : 64 * (j + 1), :], in_=st[:, :]
                        )

                def emit_s_exp(b, fc, tt):
                    sp = s_ps_pool.tile([128, 2, 512], F32, tag="s")
                    for j in range(2):
                        nc.tensor.matmul(
                            sp[:, j, :],
                            lhsT=kT_sb[
                                64 * j : 64 * (j + 1), b, 128 * tt : 128 * (tt + 1)
                            ],
                            rhs=qTp_sb[
                                64 * j : 64 * (j + 1), b, 512 * fc : 512 * (fc + 1)
                            ],
                            start=True,
                            stop=True,
                        )
                    pt = pt_pool.tile([128, 2, 512], BF16, tag="pt")
                    nc.scalar.activation(
                        out=pt[:, :, :],
                        in_=sp[:, :, :],
                        func=mybir.ActivationFunctionType.Exp,
                        scale=float(DH) ** -0.5,
                    )
                    return pt

                # software-pipelined: S/exp run one (b,fc,tt) step ahead of
                # the P@V accumulation so the in-order PE never waits on exp
                steps = [
                    (b, fc, tt) for b in range(B) for fc in range(FC)
                    for tt in range(TT)
                ]
                pending = None
                a_tiles = {}
                pts = {}
                pts[steps[0]] = emit_s_exp(*steps[0])
                for i, (b, fc, tt) in enumerate(steps):
                    if tt == 0:
                        a_tiles[(b, fc)] = a_ps_pool.tile(
                            [65, 2, 512], F32, tag="a", name="a_acc"
                        )
                    if i + 1 < len(steps):
                        pts[steps[i + 1]] = emit_s_exp(*steps[i + 1])
                    a_ps = a_tiles[(b, fc)]
                    pt = pts.pop((b, fc, tt))
                    for j in range(2):
                        nc.tensor.matmul(
                            a_ps[:, j, :],
                            lhsT=v_sb[:, b, tt, j, :],
                            rhs=pt[:, j, :],
                            start=(tt == 0),
                            stop=(tt == TT - 1),
                        )
                    if tt == 8 and pending is not None:
                        flush_scale(*pending)
                        pending = None
                    if tt == TT - 1:
                        pending = (b, fc, a_ps)
                flush_scale(*pending)

        nc.scalar.dma_start(
            out=wo_sb[:, :, :], in_=wo[:, :].rearrange("(a p) n -> p a n", p=128)
        )

        # ---- AllToAll: head-sharded -> (batch, f-slice)-sharded -------
        nc.gpsimd.collective_compute(
            "AllToAll",
            mybir.AluOpType.bypass,
            replica_groups=[[0, 1, 2, 3, 4, 5, 6, 7]],
            ins=[a2a_in.ap().opt()],
            outs=[a2a_out.ap().opt()],
        )

        with (
            tc.tile_pool(name="atg", bufs=1) as atg_pool,
            tc.tile_pool(name="o_ps", bufs=4, space="PSUM") as o_ps_pool,
            tc.tile_pool(name="op", bufs=2) as out_pool,
        ):
            atg_sb = atg_pool.tile([128, HT, FS], BF16, tag="atg")
            nc.sync.dma_start(
                out=atg_sb[:, :, :],
                in_=a2a_out[:, :].rearrange("a (p n) -> p a n", p=128),
            )
            for ft in range(FS // 128):
                o_ps = o_ps_pool.tile([128, 2, 512], F32, tag="o")
                for p in range(HT):
                    for j in range(2):
                        nc.tensor.matmul(
                            o_ps[:, j, :],
                            lhsT=atg_sb[:, p, 128 * ft : 128 * (ft + 1)],
                            rhs=wo_sb[:, p, 512 * j : 512 * (j + 1)],
                            start=(p == 0),
                            stop=(p == HT - 1),
                        )
                ot = out_pool.tile([128, HID], F32, tag="ot")
                nc.vector.tensor_copy(
                    out=ot[:, :].rearrange("p (j n) -> p j n", j=2),
                    in_=o_ps[:, :, :],
                )
                nc.sync.dma_start(
                    out=out[128 * ft : 128 * (ft + 1), :], in_=ot[:, :]
                )

    nc.compile()
    return nc


def _get_nc():
    if "nc" not in _CACHE:
        _CACHE["nc"] = _build()
    return _CACHE["nc"]


def _reference_fallback(query_input, source_input, bias, wq, wk, wv, wo):
    """Numpy fallback, only used if bias is unexpectedly nonzero."""
    q = np.einsum("bfh,hnd->bfnd", query_input, wq) * (DH**-0.5)
    k = np.einsum("bth,hnd->btnd", source_input, wk)
    v = np.einsum("bth,hnd->btnd", source_input, wv)
    logits = np.einsum("btnd,bfnd->bnft", k, q) + bias
    logits -= logits.max(axis=-1, keepdims=True)
    w = np.exp(logits)
    w /= w.sum(axis=-1, keepdims=True)
    attn = np.einsum("bnft,btnd->bfnd", w, v)
    return np.einsum("bfnd,ndh->bfh", attn, wo).astype(np.float32)


def make_in_maps(query_input, source_input, wq, wk, wv, wo):
    wo2 = np.ascontiguousarray(wo.reshape(HID, HID).astype(NPBF16))
    qTb = np.ascontiguousarray(
        np.transpose(query_input, (0, 2, 1))
    ).astype(NPBF16)  # [B, HID, F]
    sTb = np.ascontiguousarray(np.transpose(source_input, (0, 2, 1))).astype(NPBF16)
    wqh = wq.reshape(HID, NH, DH)
    wkh = wk.reshape(HID, NH, DH)
    wvh = wv.reshape(HID, NH, DH)

    in_maps = []
    for c in range(8):
        sl = np.s_[:, 2 * c : 2 * c + 2, :]
        in_maps.append(
            {
                "qT": qTb,
                "sT": sTb,
                "wq": np.ascontiguousarray(wqh[sl].reshape(HID, 128)).astype(NPBF16),
                "wk": np.ascontiguousarray(wkh[sl].reshape(HID, 128)).astype(NPBF16),
                "wv": np.ascontiguousarray(wvh[sl].reshape(HID, 128)).astype(NPBF16),
                "wo": wo2,
            }
        )
    return in_maps


def kernel(query_input, source_input, bias, wq, wk, wv, wo):
    query_input = np.asarray(query_input, dtype=np.float32)
    source_input = np.asarray(source_input, dtype=np.float32)
    bias = np.asarray(bias, dtype=np.float32)
    wq = np.asarray(wq, dtype=np.float32)
    wk = np.asarray(wk, dtype=np.float32)
    wv = np.asarray(wv, dtype=np.float32)
    wo = np.asarray(wo, dtype=np.float32)

    if np.any(bias):
        return _reference_fallback(query_input, source_input, bias, wq, wk, wv, wo)

    in_maps = make_in_maps(query_input, source_input, wq, wk, wv, wo)
    nc = _get_nc()
    res = run_bass_kernel_spmd(nc, in_maps, core_ids=list(range(8)))

    out_full = np.empty((B, F, HID), dtype=np.float32)
    for c in range(8):
        b, r = c // 4, c % 4
        out_full[b, FS * r : FS * (r + 1), :] = res.results[c]["out"]
    return out_full


# revision 30
# speedup vs baseline: 1.1315x; 1.1315x over previous
"""Multi-head attention (B=2, F=T=2048, H=1024, 16 heads x 64) on 8 TRN2
NeuronCores.

Sharding (v2): pure head/tensor parallelism with an output-side AllToAll.
Core c owns heads {2c, 2c+1} for BOTH batches. Each core:
  1. projects Q^T / K^T / V for its 2 heads over the full sequences,
  2. runs attention for its heads (softmax denominators come free from a
     ones-column appended to V in the P@V matmul; exp runs on the scalar
     engine with the 1/sqrt(64) logit scale folded into its free affine),
  3. normalizes A^T off the PE (reciprocal_approx_fast on DVE + gpsimd
     partition_broadcast), then
  4. one 8-core AllToAll redistributes A^T from head-sharded to
     (batch, query-slice)-sharded, and the output projection runs locally
     with the full 1024-deep head contraction -> exact [512, 1024] slice.
Host concatenates the 8 slices. All matmuls run in bf16 with fp32 PSUM
accumulation.
"""

from contextlib import ExitStack

import ml_dtypes
import numpy as np

import concourse.bass as bass  # noqa: F401
import concourse.mybir as mybir
import concourse.tile as tile
from concourse import bacc
from concourse.bass_utils import run_bass_kernel_spmd

B, F, T, HID, NH, DH = 2, 2048, 2048, 1024, 16, 64
FS = F // 4  # 512-row output slice per core
HT = HID // 128  # 8 h-tiles
TT = T // 128  # 16 key tiles
FC = F // 512  # 4 query chunks
BF16, F32 = mybir.dt.bfloat16, mybir.dt.float32
NPBF16 = ml_dtypes.bfloat16

_CACHE: dict = {}


def _build():
    nc = bacc.Bacc("TRN2", target_bir_lowering=False, debug=False, num_devices=8)

    qT = nc.declare_dram_parameter("qT", [B, HID, F], BF16, isOutput=False)
    sT = nc.declare_dram_parameter("sT", [B, HID, T], BF16, isOutput=False)
    wq = nc.declare_dram_parameter("wq", [HID, 128], BF16, isOutput=False)
    wk = nc.declare_dram_parameter("wk", [HID, 128], BF16, isOutput=False)
    wv = nc.declare_dram_parameter("wv", [HID, 128], BF16, isOutput=False)
    wo = nc.declare_dram_parameter("wo", [HID, HID], BF16, isOutput=False)
    out = nc.declare_dram_parameter("out", [FS, HID], F32, isOutput=True)

    seg = 128 * FS  # one A^T shard: [128 hd, 512 f]
    a2a_in = nc.dram_tensor("a2a_in", [8, seg], BF16)
    a2a_out = nc.dram_tensor("a2a_out", [8, seg], BF16)

    with tile.TileContext(nc) as tc, ExitStack() as ctx:
        persist = ctx.enter_context(tc.tile_pool(name="persist", bufs=1))
        kT_sb = persist.tile([128, B, T], BF16, tag="kT")
        v_sb = persist.tile([128, B, TT, 2, DH + 1], BF16, tag="v")
        qTp_sb = persist.tile([128, B, F], BF16, tag="qTp")
        wo_sb = persist.tile([128, HT, HID], BF16, tag="wo")
        w3_sb = persist.tile([128, HT, 3, 128], BF16, tag="w3")  # wq|wk|wv
        ones_sb = persist.tile([128, DH, 1], BF16, tag="ones")

        nc.vector.memset(ones_sb[:, :, :], 1.0)
        nc.vector.memset(v_sb[:, :, :, :, DH : DH + 1], 1.0)
        nc.sync.dma_start(
            out=w3_sb[:, :, 0, :], in_=wq[:, :].rearrange("(a p) n -> p a n", p=128)
        )
        nc.sync.dma_start(
            out=w3_sb[:, :, 1, :], in_=wk[:, :].rearrange("(a p) n -> p a n", p=128)
        )
        nc.sync.dma_start(
            out=w3_sb[:, :, 2, :], in_=wv[:, :].rearrange("(a p) n -> p a n", p=128)
        )

        with (
            tc.tile_pool(name="inp", bufs=1) as inp_pool,
            tc.tile_pool(name="inps", bufs=2) as inps_pool,
            tc.tile_pool(name="ptp", bufs=4) as pt_pool,
            tc.tile_pool(name="rtp", bufs=2) as rt_pool,
            tc.tile_pool(name="stg", bufs=2) as stg_pool,
        ):
            # ---- projections (both batches, PSUM scope closes after) --
            with tc.tile_pool(name="proj_ps", bufs=2, space="PSUM") as proj_ps:
                qT_tiles = []
                for b in range(B):
                    sT_sb = inps_pool.tile([128, HT, T], BF16, tag="sT")
                    nc.sync.dma_start(
                        out=sT_sb[:, :, :],
                        in_=sT[b, :, :].rearrange("(a p) n -> p a n", p=128),
                    )
                    # K^T [128 hd, T]
                    for c in range(T // 512):
                        ps = proj_ps.tile([128, 512], F32, tag="ps")
                        for ht in range(HT):
                            nc.tensor.matmul(
                                ps[:, :],
                                lhsT=w3_sb[:, ht, 1, :],
                                rhs=sT_sb[:, ht, 512 * c : 512 * (c + 1)],
                                start=(ht == 0),
                                stop=(ht == HT - 1),
                            )
                        nc.vector.tensor_copy(
                            out=kT_sb[:, b, 512 * c : 512 * (c + 1)], in_=ps[:, :]
                        )
                    # V [t, 2*DH] per key tile
                    for tt in range(TT):
                        ps = proj_ps.tile([128, 128], F32, tag="ps")
                        for ht in range(HT):
                            nc.tensor.matmul(
                                ps[:, :],
                                lhsT=sT_sb[:, ht, 128 * tt : 128 * (tt + 1)],
                                rhs=w3_sb[:, ht, 2, :],
                                start=(ht == 0),
                                stop=(ht == HT - 1),
                            )
                        nc.vector.tensor_copy(
                            out=v_sb[:, b, tt, :, 0:DH],
                            in_=ps[:, :].rearrange("p (j d) -> p j d", j=2),
                        )
                    # Q^T [128 hd, F]
                    qT_sb = inp_pool.tile([128, HT, F], BF16, tag="qT")
                    nc.scalar.dma_start(
                        out=qT_sb[:, :, :],
                        in_=qT[b, :, :].rearrange("(a p) n -> p a n", p=128),
                    )
                    for c in range(FC):
                        ps = proj_ps.tile([128, 512], F32, tag="ps")
                        for ht in range(HT):
                            nc.tensor.matmul(
                                ps[:, :],
                                lhsT=w3_sb[:, ht, 0, :],
                                rhs=qT_sb[:, ht, 512 * c : 512 * (c + 1)],
                                start=(ht == 0),
                                stop=(ht == HT - 1),
                            )
                        nc.vector.tensor_copy(
                            out=qTp_sb[:, b, 512 * c : 512 * (c + 1)], in_=ps[:, :]
                        )

            # ---- attention (both batches); scale chains deferred by one
            # fc so the in-order PE never stalls on recip/broadcast ------
            with (
                tc.tile_pool(name="s_ps", bufs=2, space="PSUM") as s_ps_pool,
                tc.tile_pool(name="a_ps", bufs=2, space="PSUM") as a_ps_pool,
            ):

                def flush_scale(b, fc, a_ps):
                    shard = a2a_in[4 * b + fc, :].rearrange(
                        "(p n) -> p n", p=128
                    )
                    rts = []
                    for j in range(2):
                        rt = rt_pool.tile([65, 1, 512], BF16, tag="rt")
                        with nc.allow_low_precision("bf16 softmax denom recip"):
                            nc.vector.reciprocal(
                                out=rt[64:65, 0, :], in_=a_ps[64:65, j, :]
                            )
                        rts.append(rt)
                    for j in range(2):
                        rt = rts[j]
                        bc = s_ps_pool.tile([64, 512], F32, tag="s")
                        nc.tensor.matmul(
                            bc[:, :],
                            lhsT=ones_sb[64:65, :, 0],
                            rhs=rt[64:65, 0, :],
                            start=True,
                            stop=True,
                        )
                        bc_sb = rt_pool.tile([64, 512], F32, tag="bc")
                        nc.vector.tensor_copy(out=bc_sb[:, :], in_=bc[:, :])
                        st = stg_pool.tile([64, 512], BF16, tag="st")
                        nc.vector.tensor_mul(
                            out=st[:, :], in0=a_ps[0:64, j, :], in1=bc_sb[:, :]
                        )
                        nc.sync.dma_start(
                            out=shard[64 * j : 64 * (j + 1), :], in_=st[:, :]
                        )

                def emit_s_exp(b, fc, tt):
                    sp = s_ps_pool.tile([128, 2, 512], F32, tag="s")
                    for j in range(2):
                        nc.tensor.matmul(
                            sp[:, j, :],
                            lhsT=kT_sb[
                                64 * j : 64 * (j + 1), b, 128 * tt : 128 * (tt + 1)
                            ],
                            rhs=qTp_sb[
                                64 * j : 64 * (j + 1), b, 512 * fc : 512 * (fc + 1)
                            ],
                            start=True,
                            stop=True,
                        )
                    pt = pt_pool.tile([128, 2, 512], BF16, tag="pt")
                    nc.scalar.activation(
                        out=pt[:, :, :],
                        in_=sp[:, :, :],
                        func=mybir.ActivationFunctionType.Exp,
                        scale=float(DH) ** -0.5,
                    )
                    return pt

                # software-pipelined: S/exp run one (b,fc,tt) step ahead of
                # the P@V accumulation so the in-order PE never waits on exp
                steps = [
                    (b, fc, tt) for b in range(B) for fc in range(FC)
                    for tt in range(TT)
                ]
                pending = None
                a_tiles = {}
                pts = {}
                pts[steps[0]] = emit_s_exp(*steps[0])
                for i, (b, fc, tt) in enumerate(steps):
                    if tt == 0:
                        a_tiles[(b, fc)] = a_ps_pool.tile(
                            [65, 2, 512], F32, tag="a", name="a_acc"
                        )
                    if i + 1 < len(steps):
                        pts[steps[i + 1]] = emit_s_exp(*steps[i + 1])
                    a_ps = a_tiles[(b, fc)]
                    pt = pts.pop((b, fc, tt))
                    for j in range(2):
                        nc.tensor.matmul(
                            a_ps[:, j, :],
                            lhsT=v_sb[:, b, tt, j, :],
                            rhs=pt[:, j, :],
                            start=(tt == 0),
                            stop=(tt == TT - 1),
                        )
                    if tt == 8 and pending is not None:
                        flush_scale(*pending)
                        pending = None
                    if tt == TT - 1:
                        pending = (b, fc, a_ps)
                flush_scale(*pending)

        nc.scalar.dma_start(
            out=wo_sb[:, :, :], in_=wo[:, :].rearrange("(a p) n -> p a n", p=128)
        )

        # ---- AllToAll: head-sharded -> (batch, f-slice)-sharded -------
        nc.gpsimd.collective_compute(
            "AllToAll",
            mybir.AluOpType.bypass,
            replica_groups=[[0, 1, 2, 3, 4, 5, 6, 7]],
            ins=[a2a_in.ap().opt()],
            outs=[a2a_out.ap().opt()],
        )

        with (
            tc.tile_pool(name="atg", bufs=1) as atg_pool,
            tc.tile_pool(name="o_ps", bufs=4, space="PSUM") as o_ps_pool,
            tc.tile_pool(name="op", bufs=2) as out_pool,
        ):
            atg_sb = atg_pool.tile([128, HT, FS], BF16, tag="atg")
            nc.sync.dma_start(
                out=atg_sb[:, :, :],
                in_=a2a_out[:, :].rearrange("a (p n) -> p a n", p=128),
            )
            for ft in range(FS // 128):
                o_ps = o_ps_pool.tile([128, 2, 512], F32, tag="o")
                for p in range(HT):
                    for j in range(2):
                        nc.tensor.matmul(
                            o_ps[:, j, :],
                            lhsT=atg_sb[:, p, 128 * ft : 128 * (ft + 1)],
                            rhs=wo_sb[:, p, 512 * j : 512 * (j + 1)],
                            start=(p == 0),
                            stop=(p == HT - 1),
                        )
                ot = out_pool.tile([128, HID], F32, tag="ot")
                nc.vector.tensor_copy(
                    out=ot[:, :].rearrange("p (j n) -> p j n", j=2),
                    in_=o_ps[:, :, :],
                )
                nc.sync.dma_start(
                    out=out[128 * ft : 128 * (ft + 1), :], in_=ot[:, :]
                )

    nc.compile()
    return nc


def _get_nc():
    if "nc" not in _CACHE:
        _CACHE["nc"] = _build()
    return _CACHE["nc"]


def _reference_fallback(query_input, source_input, bias, wq, wk, wv, wo):
    """Numpy fallback, only used if bias is unexpectedly nonzero."""
    q = np.einsum("bfh,hnd->bfnd", query_input, wq) * (DH**-0.5)
    k = np.einsum("bth,hnd->btnd", source_input, wk)
    v = np.einsum("bth,hnd->btnd", source_input, wv)
    logits = np.einsum("btnd,bfnd->bnft", k, q) + bias
    logits -= logits.max(axis=-1, keepdims=True)
    w = np.exp(logits)
    w /= w.sum(axis=-1, keepdims=True)
    attn = np.einsum("bnft,btnd->bfnd", w, v)
    return np.einsum("bfnd,ndh->bfh", attn, wo).astype(np.float32)


def make_in_maps(query_input, source_input, wq, wk, wv, wo):
    wo2 = np.ascontiguousarray(wo.reshape(HID, HID).astype(NPBF16))
    qTb = np.ascontiguousarray(
        np.transpose(query_input, (0, 2, 1))
    ).astype(NPBF16)  # [B, HID, F]
    sTb = np.ascontiguousarray(np.transpose(source_input, (0, 2, 1))).astype(NPBF16)
    wqh = wq.reshape(HID, NH, DH)
    wkh = wk.reshape(HID, NH, DH)
    wvh = wv.reshape(HID, NH, DH)

    in_maps = []
    for c in range(8):
        sl = np.s_[:, 2 * c : 2 * c + 2, :]
        in_maps.append(
            {
                "qT": qTb,
                "sT": sTb,
                "wq": np.ascontiguousarray(wqh[sl].reshape(HID, 128)).astype(NPBF16),
                "wk": np.ascontiguousarray(wkh[sl].reshape(HID, 128)).astype(NPBF16),
                "wv": np.ascontiguousarray(wvh[sl].reshape(HID, 128)).astype(NPBF16),
                "wo": wo2,
            }
        )
    return in_maps


def kernel(query_input, source_input, bias, wq, wk, wv, wo):
    query_input = np.asarray(query_input, dtype=np.float32)
    source_input = np.asarray(source_input, dtype=np.float32)
    bias = np.asarray(bias, dtype=np.float32)
    wq = np.asarray(wq, dtype=np.float32)
    wk = np.asarray(wk, dtype=np.float32)
    wv = np.asarray(wv, dtype=np.float32)
    wo = np.asarray(wo, dtype=np.float32)

    if np.any(bias):
        return _reference_fallback(query_input, source_input, bias, wq, wk, wv, wo)

    in_maps = make_in_maps(query_input, source_input, wq, wk, wv, wo)
    nc = _get_nc()
    res = run_bass_kernel_spmd(nc, in_maps, core_ids=list(range(8)))

    out_full = np.empty((B, F, HID), dtype=np.float32)
    for c in range(8):
        b, r = c // 4, c % 4
        out_full[b, FS * r : FS * (r + 1), :] = res.results[c]["out"]
    return out_full


# revision 31
# speedup vs baseline: 1.1335x; 1.0018x over previous
"""Multi-head attention (B=2, F=T=2048, H=1024, 16 heads x 64) on 8 TRN2
NeuronCores.

Sharding: pure head/tensor parallelism with an output-side AllToAll.
Core c owns heads {2c, 2c+1} for BOTH batches. Each core:
  1. projects Q^T / K^T / V for its 2 heads over the full sequences
     (both batches, front-loaded so attention owns all 8 PSUM banks),
  2. runs attention for its heads, software-pipelined so the ACT engine
     (exp) is the critical path: S/exp for step i+1 are emitted before
     the P@V matmuls of step i. Softmax denominators come free from a
     ones-column appended to V in the P@V matmul; exp folds the
     1/sqrt(64) logit scale into its free affine,
  3. normalizes A^T with a scale chain (DVE reciprocal -> Kc=1 PE
     ones-broadcast -> DVE multiply) deferred into the next f-chunk's
     loop so the in-order PE never stalls on it; each finished (batch,
     f-chunk) shard is DMA'd straight into the AllToAll input buffer,
  4. one 8-core AllToAll redistributes A^T from head-sharded to
     (batch, query-slice)-sharded, and the output projection runs locally
     with the full 1024-deep head contraction -> exact [512, 1024] slice.
Host concatenates the 8 slices. All matmuls run in bf16 with fp32 PSUM
accumulation.
"""

from contextlib import ExitStack

import ml_dtypes
import numpy as np

import concourse.bass as bass  # noqa: F401
import concourse.mybir as mybir
import concourse.tile as tile
from concourse import bacc
from concourse.bass_utils import run_bass_kernel_spmd

B, F, T, HID, NH, DH = 2, 2048, 2048, 1024, 16, 64
FS = F // 4  # 512-row output slice per core
HT = HID // 128  # 8 h-tiles
TT = T // 128  # 16 key tiles
FC = F // 512  # 4 query chunks
BF16, F32 = mybir.dt.bfloat16, mybir.dt.float32
NPBF16 = ml_dtypes.bfloat16

_CACHE: dict = {}


def _build():
    nc = bacc.Bacc("TRN2", target_bir_lowering=False, debug=False, num_devices=8)

    qT = nc.declare_dram_parameter("qT", [B, HID, F], BF16, isOutput=False)
    sT = nc.declare_dram_parameter("sT", [B, HID, T], BF16, isOutput=False)
    wq = nc.declare_dram_parameter("wq", [HID, 128], BF16, isOutput=False)
    wk = nc.declare_dram_parameter("wk", [HID, 128], BF16, isOutput=False)
    wv = nc.declare_dram_parameter("wv", [HID, 128], BF16, isOutput=False)
    wo = nc.declare_dram_parameter("wo", [HID, HID], BF16, isOutput=False)
    out = nc.declare_dram_parameter("out", [FS, HID], F32, isOutput=True)

    seg = 128 * FS  # one A^T shard: [128 hd, 512 f]
    a2a_in = nc.dram_tensor("a2a_in", [8, seg], BF16)
    a2a_out = nc.dram_tensor("a2a_out", [8, seg], BF16)

    with tile.TileContext(nc) as tc, ExitStack() as ctx:
        persist = ctx.enter_context(tc.tile_pool(name="persist", bufs=1))
        kT_sb = persist.tile([128, B, T], BF16, tag="kT")
        v_sb = persist.tile([128, B, TT, 2, DH + 1], BF16, tag="v")
        qTp_sb = persist.tile([128, B, F], BF16, tag="qTp")
        wo_sb = persist.tile([128, HT, HID], BF16, tag="wo")
        w3_sb = persist.tile([128, HT, 3, 128], BF16, tag="w3")  # wq|wk|wv
        ones_sb = persist.tile([128, DH, 1], BF16, tag="ones")

        nc.vector.memset(ones_sb[:, :, :], 1.0)
        nc.vector.memset(v_sb[:, :, :, :, DH : DH + 1], 1.0)
        nc.sync.dma_start(
            out=w3_sb[:, :, 0, :], in_=wq[:, :].rearrange("(a p) n -> p a n", p=128)
        )
        nc.sync.dma_start(
            out=w3_sb[:, :, 1, :], in_=wk[:, :].rearrange("(a p) n -> p a n", p=128)
        )
        nc.sync.dma_start(
            out=w3_sb[:, :, 2, :], in_=wv[:, :].rearrange("(a p) n -> p a n", p=128)
        )

        with (
            tc.tile_pool(name="inp", bufs=1) as inp_pool,
            tc.tile_pool(name="inps", bufs=2) as inps_pool,
            tc.tile_pool(name="ptp", bufs=4) as pt_pool,
            tc.tile_pool(name="rtp", bufs=2) as rt_pool,
            tc.tile_pool(name="stg", bufs=2) as stg_pool,
        ):
            # ---- projections (both batches, PSUM scope closes after) --
            with tc.tile_pool(name="proj_ps", bufs=2, space="PSUM") as proj_ps:
                for b in range(B):
                    sT_sb = inps_pool.tile([128, HT, T], BF16, tag="sT")
                    nc.sync.dma_start(
                        out=sT_sb[:, :, :],
                        in_=sT[b, :, :].rearrange("(a p) n -> p a n", p=128),
                    )
                    # K^T [128 hd, T]
                    for c in range(T // 512):
                        ps = proj_ps.tile([128, 512], F32, tag="ps")
                        for ht in range(HT):
                            nc.tensor.matmul(
                                ps[:, :],
                                lhsT=w3_sb[:, ht, 1, :],
                                rhs=sT_sb[:, ht, 512 * c : 512 * (c + 1)],
                                start=(ht == 0),
                                stop=(ht == HT - 1),
                            )
                        nc.vector.tensor_copy(
                            out=kT_sb[:, b, 512 * c : 512 * (c + 1)], in_=ps[:, :]
                        )
                    # V [t, 2*DH] per key tile
                    for tt in range(TT):
                        ps = proj_ps.tile([128, 128], F32, tag="ps")
                        for ht in range(HT):
                            nc.tensor.matmul(
                                ps[:, :],
                                lhsT=sT_sb[:, ht, 128 * tt : 128 * (tt + 1)],
                                rhs=w3_sb[:, ht, 2, :],
                                start=(ht == 0),
                                stop=(ht == HT - 1),
                            )
                        nc.vector.tensor_copy(
                            out=v_sb[:, b, tt, :, 0:DH],
                            in_=ps[:, :].rearrange("p (j d) -> p j d", j=2),
                        )
                    # Q^T [128 hd, F]
                    qT_sb = inp_pool.tile([128, HT, F], BF16, tag="qT")
                    nc.scalar.dma_start(
                        out=qT_sb[:, :, :],
                        in_=qT[b, :, :].rearrange("(a p) n -> p a n", p=128),
                    )
                    for c in range(FC):
                        ps = proj_ps.tile([128, 512], F32, tag="ps")
                        for ht in range(HT):
                            nc.tensor.matmul(
                                ps[:, :],
                                lhsT=w3_sb[:, ht, 0, :],
                                rhs=qT_sb[:, ht, 512 * c : 512 * (c + 1)],
                                start=(ht == 0),
                                stop=(ht == HT - 1),
                            )
                        nc.vector.tensor_copy(
                            out=qTp_sb[:, b, 512 * c : 512 * (c + 1)], in_=ps[:, :]
                        )

            # ---- attention (both batches); scale chains deferred by one
            # fc so the in-order PE never stalls on recip/broadcast ------
            with (
                tc.tile_pool(name="s_ps", bufs=2, space="PSUM") as s_ps_pool,
                tc.tile_pool(name="a_ps", bufs=2, space="PSUM") as a_ps_pool,
            ):

                def flush_scale(b, fc, a_ps):
                    shard = a2a_in[4 * b + fc, :].rearrange(
                        "(p n) -> p n", p=128
                    )
                    rts = []
                    for j in range(2):
                        rt = rt_pool.tile([65, 1, 512], BF16, tag="rt")
                        with nc.allow_low_precision("bf16 softmax denom recip"):
                            nc.vector.reciprocal(
                                out=rt[64:65, 0, :], in_=a_ps[64:65, j, :]
                            )
                        rts.append(rt)
                    for j in range(2):
                        rt = rts[j]
                        bc = s_ps_pool.tile([64, 512], F32, tag="s")
                        nc.tensor.matmul(
                            bc[:, :],
                            lhsT=ones_sb[64:65, :, 0],
                            rhs=rt[64:65, 0, :],
                            start=True,
                            stop=True,
                        )
                        bc_sb = rt_pool.tile([64, 512], F32, tag="bc")
                        nc.vector.tensor_copy(out=bc_sb[:, :], in_=bc[:, :])
                        st = stg_pool.tile([64, 512], BF16, tag="st")
                        nc.vector.tensor_mul(
                            out=st[:, :], in0=a_ps[0:64, j, :], in1=bc_sb[:, :]
                        )
                        nc.sync.dma_start(
                            out=shard[64 * j : 64 * (j + 1), :], in_=st[:, :]
                        )

                def emit_s_exp(b, fc, tt):
                    sp = s_ps_pool.tile([128, 2, 512], F32, tag="s")
                    for j in range(2):
                        nc.tensor.matmul(
                            sp[:, j, :],
                            lhsT=kT_sb[
                                64 * j : 64 * (j + 1), b, 128 * tt : 128 * (tt + 1)
                            ],
                            rhs=qTp_sb[
                                64 * j : 64 * (j + 1), b, 512 * fc : 512 * (fc + 1)
                            ],
                            start=True,
                            stop=True,
                        )
                    pt = pt_pool.tile([128, 2, 512], BF16, tag="pt")
                    nc.scalar.activation(
                        out=pt[:, :, :],
                        in_=sp[:, :, :],
                        func=mybir.ActivationFunctionType.Exp,
                        scale=float(DH) ** -0.5,
                    )
                    return pt

                # software-pipelined: S/exp run one (b,fc,tt) step ahead of
                # the P@V accumulation so the in-order PE never waits on exp
                steps = [
                    (b, fc, tt) for b in range(B) for fc in range(FC)
                    for tt in range(TT)
                ]
                pending = None
                a_tiles = {}
                pts = {}
                pts[steps[0]] = emit_s_exp(*steps[0])
                for i, (b, fc, tt) in enumerate(steps):
                    if tt == 0:
                        a_tiles[(b, fc)] = a_ps_pool.tile(
                            [65, 2, 512], F32, tag="a", name="a_acc"
                        )
                    if i + 1 < len(steps):
                        pts[steps[i + 1]] = emit_s_exp(*steps[i + 1])
                    a_ps = a_tiles[(b, fc)]
                    pt = pts.pop((b, fc, tt))
                    for j in range(2):
                        nc.tensor.matmul(
                            a_ps[:, j, :],
                            lhsT=v_sb[:, b, tt, j, :],
                            rhs=pt[:, j, :],
                            start=(tt == 0),
                            stop=(tt == TT - 1),
                        )
                    if tt == 8 and pending is not None:
                        flush_scale(*pending)
                        pending = None
                    if tt == TT - 1:
                        pending = (b, fc, a_ps)
                flush_scale(*pending)

        nc.scalar.dma_start(
            out=wo_sb[:, :, :], in_=wo[:, :].rearrange("(a p) n -> p a n", p=128)
        )

        # ---- AllToAll: head-sharded -> (batch, f-slice)-sharded -------
        nc.gpsimd.collective_compute(
            "AllToAll",
            mybir.AluOpType.bypass,
            replica_groups=[[0, 1, 2, 3, 4, 5, 6, 7]],
            ins=[a2a_in.ap().opt()],
            outs=[a2a_out.ap().opt()],
        )

        with (
            tc.tile_pool(name="atg", bufs=1) as atg_pool,
            tc.tile_pool(name="o_ps", bufs=4, space="PSUM") as o_ps_pool,
            tc.tile_pool(name="op", bufs=2) as out_pool,
        ):
            atg_sb = atg_pool.tile([128, HT, FS], BF16, tag="atg")
            nc.sync.dma_start(
                out=atg_sb[:, :, :],
                in_=a2a_out[:, :].rearrange("a (p n) -> p a n", p=128),
            )
            for ft in range(FS // 128):
                o_ps = o_ps_pool.tile([128, 2, 512], F32, tag="o")
                for p in range(HT):
                    for j in range(2):
                        nc.tensor.matmul(
                            o_ps[:, j, :],
                            lhsT=atg_sb[:, p, 128 * ft : 128 * (ft + 1)],
                            rhs=wo_sb[:, p, 512 * j : 512 * (j + 1)],
                            start=(p == 0),
                            stop=(p == HT - 1),
                        )
                ot = out_pool.tile([128, HID], F32, tag="ot")
                nc.vector.tensor_copy(
                    out=ot[:, :].rearrange("p (j n) -> p j n", j=2),
                    in_=o_ps[:, :, :],
                )
                nc.sync.dma_start(
                    out=out[128 * ft : 128 * (ft + 1), :], in_=ot[:, :]
                )

    nc.compile()
    return nc


def _get_nc():
    if "nc" not in _CACHE:
        _CACHE["nc"] = _build()
    return _CACHE["nc"]


def _reference_fallback(query_input, source_input, bias, wq, wk, wv, wo):
    """Numpy fallback, only used if bias is unexpectedly nonzero."""
    q = np.einsum("bfh,hnd->bfnd", query_input, wq) * (DH**-0.5)
    k = np.einsum("bth,hnd->btnd", source_input, wk)
    v = np.einsum("bth,hnd->btnd", source_input, wv)
    logits = np.einsum("btnd,bfnd->bnft", k, q) + bias
    logits -= logits.max(axis=-1, keepdims=True)
    w = np.exp(logits)
    w /= w.sum(axis=-1, keepdims=True)
    attn = np.einsum("bnft,btnd->bfnd", w, v)
    return np.einsum("bfnd,ndh->bfh", attn, wo).astype(np.float32)


def make_in_maps(query_input, source_input, wq, wk, wv, wo):
    wo2 = np.ascontiguousarray(wo.reshape(HID, HID).astype(NPBF16))
    qTb = np.ascontiguousarray(
        np.transpose(query_input, (0, 2, 1))
    ).astype(NPBF16)  # [B, HID, F]
    sTb = np.ascontiguousarray(np.transpose(source_input, (0, 2, 1))).astype(NPBF16)
    wqh = wq.reshape(HID, NH, DH)
    wkh = wk.reshape(HID, NH, DH)
    wvh = wv.reshape(HID, NH, DH)

    in_maps = []
    for c in range(8):
        sl = np.s_[:, 2 * c : 2 * c + 2, :]
        in_maps.append(
            {
                "qT": qTb,
                "sT": sTb,
                "wq": np.ascontiguousarray(wqh[sl].reshape(HID, 128)).astype(NPBF16),
                "wk": np.ascontiguousarray(wkh[sl].reshape(HID, 128)).astype(NPBF16),
                "wv": np.ascontiguousarray(wvh[sl].reshape(HID, 128)).astype(NPBF16),
                "wo": wo2,
            }
        )
    return in_maps


def kernel(query_input, source_input, bias, wq, wk, wv, wo):
    query_input = np.asarray(query_input, dtype=np.float32)
    source_input = np.asarray(source_input, dtype=np.float32)
    bias = np.asarray(bias, dtype=np.float32)
    wq = np.asarray(wq, dtype=np.float32)
    wk = np.asarray(wk, dtype=np.float32)
    wv = np.asarray(wv, dtype=np.float32)
    wo = np.asarray(wo, dtype=np.float32)

    if np.any(bias):
        return _reference_fallback(query_input, source_input, bias, wq, wk, wv, wo)

    in_maps = make_in_maps(query_input, source_input, wq, wk, wv, wo)
    nc = _get_nc()
    res = run_bass_kernel_spmd(nc, in_maps, core_ids=list(range(8)))

    out_full = np.empty((B, F, HID), dtype=np.float32)
    for c in range(8):
        b, r = c // 4, c % 4
        out_full[b, FS * r : FS * (r + 1), :] = res.results[c]["out"]
    return out_full


# revision 32
# speedup vs baseline: 1.1662x; 1.0288x over previous
"""Multi-head attention (B=2, F=T=2048, H=1024, 16 heads x 64) on 8 TRN2
NeuronCores.

Sharding: pure head/tensor parallelism with an output-side AllToAll.
Core c owns heads {2c, 2c+1} for BOTH batches. Each core:
  1. projects Q^T / K^T / V for its 2 heads over the full sequences
     (both batches, front-loaded so attention owns all 8 PSUM banks),
  2. runs attention for its heads, software-pipelined so the ACT engine
     (exp) is the critical path: S/exp for step i+1 are emitted before
     the P@V matmuls of step i. Softmax denominators come free from a
     ones-column appended to V in the P@V matmul; exp folds the
     1/sqrt(64) logit scale into its free affine,
  3. normalizes A^T with a scale chain (DVE reciprocal -> Kc=1 PE
     ones-broadcast -> DVE multiply) deferred into the next f-chunk's
     loop so the in-order PE never stalls on it; each finished (batch,
     f-chunk) shard is DMA'd straight into the AllToAll input buffer,
  4. one 8-core AllToAll redistributes A^T from head-sharded to
     (batch, query-slice)-sharded, and the output projection runs locally
     with the full 1024-deep head contraction -> exact [512, 1024] slice.
Host concatenates the 8 slices. All matmuls run in bf16 with fp32 PSUM
accumulation.
"""

from contextlib import ExitStack

import ml_dtypes
import numpy as np

import concourse.bass as bass  # noqa: F401
import concourse.mybir as mybir
import concourse.tile as tile
from concourse import bacc
from concourse.bass_utils import run_bass_kernel_spmd

B, F, T, HID, NH, DH = 2, 2048, 2048, 1024, 16, 64
FS = F // 4  # 512-row output slice per core
HT = HID // 128  # 8 h-tiles
TT = T // 128  # 16 key tiles
FC = F // 512  # 4 query chunks
BF16, F32 = mybir.dt.bfloat16, mybir.dt.float32
NPBF16 = ml_dtypes.bfloat16

_CACHE: dict = {}


def _build():
    nc = bacc.Bacc("TRN2", target_bir_lowering=False, debug=False, num_devices=8)

    qT = nc.declare_dram_parameter("qT", [B, HID, F], BF16, isOutput=False)
    sT = nc.declare_dram_parameter("sT", [B, HID, T], BF16, isOutput=False)
    w3 = nc.declare_dram_parameter("w3", [HID, 384], BF16, isOutput=False)
    wo = nc.declare_dram_parameter("wo", [HID, HID], BF16, isOutput=False)
    out = nc.declare_dram_parameter("out", [FS, HID], F32, isOutput=True)

    seg = 128 * FS  # one A^T shard: [128 hd, 512 f]
    a2a_in = nc.dram_tensor("a2a_in", [8, seg], BF16)
    a2a_out = nc.dram_tensor("a2a_out", [8, seg], BF16)

    with tile.TileContext(nc) as tc, ExitStack() as ctx:
        persist = ctx.enter_context(tc.tile_pool(name="persist", bufs=1))
        kT_sb = persist.tile([128, B, T], BF16, tag="kT")
        v_sb = persist.tile([128, B, TT, 2, DH + 1], BF16, tag="v")
        qTp_sb = persist.tile([128, B, F], BF16, tag="qTp")
        wo_sb = persist.tile([128, HT, HID], BF16, tag="wo")
        w3_sb = persist.tile([128, HT, 3, 128], BF16, tag="w3")  # wq|wk|wv
        ones_sb = persist.tile([128, DH, 1], BF16, tag="ones")

        nc.vector.memset(ones_sb[:, :, :], 1.0)
        nc.vector.memset(v_sb[:, :, :, :, DH : DH + 1], 1.0)
        nc.sync.dma_start(
            out=w3_sb[:, :, :, :],
            in_=w3[:, :].rearrange("(a p) (k n) -> p a k n", p=128, n=128),
        )

        with (
            tc.tile_pool(name="inp", bufs=1) as inp_pool,
            tc.tile_pool(name="inps", bufs=2) as inps_pool,
            tc.tile_pool(name="ptp", bufs=4) as pt_pool,
            tc.tile_pool(name="rtp", bufs=2) as rt_pool,
            tc.tile_pool(name="stg", bufs=2) as stg_pool,
        ):
            # ---- projections (both batches, PSUM scope closes after) --
            with tc.tile_pool(name="proj_ps", bufs=2, space="PSUM") as proj_ps:
                for b in range(B):
                    sT_sb = inps_pool.tile([128, HT, T], BF16, tag="sT")
                    nc.sync.dma_start(
                        out=sT_sb[:, :, :],
                        in_=sT[b, :, :].rearrange("(a p) n -> p a n", p=128),
                    )
                    # K^T [128 hd, T]
                    for c in range(T // 512):
                        ps = proj_ps.tile([128, 512], F32, tag="ps")
                        for ht in range(HT):
                            nc.tensor.matmul(
                                ps[:, :],
                                lhsT=w3_sb[:, ht, 1, :],
                                rhs=sT_sb[:, ht, 512 * c : 512 * (c + 1)],
                                start=(ht == 0),
                                stop=(ht == HT - 1),
                            )
                        nc.vector.tensor_copy(
                            out=kT_sb[:, b, 512 * c : 512 * (c + 1)], in_=ps[:, :]
                        )
                    # V [t, 2*DH] per key tile
                    for tt in range(TT):
                        ps = proj_ps.tile([128, 128], F32, tag="ps")
                        for ht in range(HT):
                            nc.tensor.matmul(
                                ps[:, :],
                                lhsT=sT_sb[:, ht, 128 * tt : 128 * (tt + 1)],
                                rhs=w3_sb[:, ht, 2, :],
                                start=(ht == 0),
                                stop=(ht == HT - 1),
                            )
                        nc.vector.tensor_copy(
                            out=v_sb[:, b, tt, :, 0:DH],
                            in_=ps[:, :].rearrange("p (j d) -> p j d", j=2),
                        )
                    # Q^T [128 hd, F]
                    qT_sb = inp_pool.tile([128, HT, F], BF16, tag="qT")
                    nc.scalar.dma_start(
                        out=qT_sb[:, :, :],
                        in_=qT[b, :, :].rearrange("(a p) n -> p a n", p=128),
                    )
                    for c in range(FC):
                        ps = proj_ps.tile([128, 512], F32, tag="ps")
                        for ht in range(HT):
                            nc.tensor.matmul(
                                ps[:, :],
                                lhsT=w3_sb[:, ht, 0, :],
                                rhs=qT_sb[:, ht, 512 * c : 512 * (c + 1)],
                                start=(ht == 0),
                                stop=(ht == HT - 1),
                            )
                        nc.vector.tensor_copy(
                            out=qTp_sb[:, b, 512 * c : 512 * (c + 1)], in_=ps[:, :]
                        )

            # ---- attention (both batches); scale chains deferred by one
            # fc so the in-order PE never stalls on recip/broadcast ------
            with (
                tc.tile_pool(name="s_ps", bufs=2, space="PSUM") as s_ps_pool,
                tc.tile_pool(name="a_ps", bufs=2, space="PSUM") as a_ps_pool,
            ):

                def flush_scale(b, fc, a_ps):
                    shard = a2a_in[4 * b + fc, :].rearrange(
                        "(p n) -> p n", p=128
                    )
                    rts = []
                    for j in range(2):
                        rt = rt_pool.tile([65, 1, 512], BF16, tag="rt")
                        with nc.allow_low_precision("bf16 softmax denom recip"):
                            nc.vector.reciprocal(
                                out=rt[64:65, 0, :], in_=a_ps[64:65, j, :]
                            )
                        rts.append(rt)
                    for j in range(2):
                        rt = rts[j]
                        bc = s_ps_pool.tile([64, 512], F32, tag="s")
                        nc.tensor.matmul(
                            bc[:, :],
                            lhsT=ones_sb[64:65, :, 0],
                            rhs=rt[64:65, 0, :],
                            start=True,
                            stop=True,
                        )
                        bc_sb = rt_pool.tile([64, 512], F32, tag="bc")
                        nc.vector.tensor_copy(out=bc_sb[:, :], in_=bc[:, :])
                        st = stg_pool.tile([64, 512], BF16, tag="st")
                        nc.vector.tensor_mul(
                            out=st[:, :], in0=a_ps[0:64, j, :], in1=bc_sb[:, :]
                        )
                        nc.sync.dma_start(
                            out=shard[64 * j : 64 * (j + 1), :], in_=st[:, :]
                        )

                def emit_s_exp(b, fc, tt):
                    sp = s_ps_pool.tile([128, 2, 512], F32, tag="s")
                    for j in range(2):
                        nc.tensor.matmul(
                            sp[:, j, :],
                            lhsT=kT_sb[
                                64 * j : 64 * (j + 1), b, 128 * tt : 128 * (tt + 1)
                            ],
                            rhs=qTp_sb[
                                64 * j : 64 * (j + 1), b, 512 * fc : 512 * (fc + 1)
                            ],
                            start=True,
                            stop=True,
                        )
                    pt = pt_pool.tile([128, 2, 512], BF16, tag="pt")
                    nc.scalar.activation(
                        out=pt[:, :, :],
                        in_=sp[:, :, :],
                        func=mybir.ActivationFunctionType.Exp,
                        scale=float(DH) ** -0.5,
                    )
                    return pt

                # software-pipelined: S/exp run one (b,fc,tt) step ahead of
                # the P@V accumulation so the in-order PE never waits on exp
                steps = [
                    (b, fc, tt) for b in range(B) for fc in range(FC)
                    for tt in range(TT)
                ]
                pending = None
                a_tiles = {}
                pts = {}
                pts[steps[0]] = emit_s_exp(*steps[0])
                for i, (b, fc, tt) in enumerate(steps):
                    if tt == 0:
                        a_tiles[(b, fc)] = a_ps_pool.tile(
                            [65, 2, 512], F32, tag="a", name="a_acc"
                        )
                    if i + 1 < len(steps):
                        pts[steps[i + 1]] = emit_s_exp(*steps[i + 1])
                    a_ps = a_tiles[(b, fc)]
                    pt = pts.pop((b, fc, tt))
                    for j in range(2):
                        nc.tensor.matmul(
                            a_ps[:, j, :],
                            lhsT=v_sb[:, b, tt, j, :],
                            rhs=pt[:, j, :],
                            start=(tt == 0),
                            stop=(tt == TT - 1),
                        )
                    if tt == 10 and pending is not None:
                        flush_scale(*pending)
                        pending = None
                    if tt == TT - 1:
                        pending = (b, fc, a_ps)
                flush_scale(*pending)

        nc.scalar.dma_start(
            out=wo_sb[:, :, :], in_=wo[:, :].rearrange("(a p) n -> p a n", p=128)
        )

        # ---- AllToAll: head-sharded -> (batch, f-slice)-sharded -------
        nc.gpsimd.collective_compute(
            "AllToAll",
            mybir.AluOpType.bypass,
            replica_groups=[[0, 1, 2, 3, 4, 5, 6, 7]],
            ins=[a2a_in.ap().opt()],
            outs=[a2a_out.ap().opt()],
        )

        with (
            tc.tile_pool(name="atg", bufs=1) as atg_pool,
            tc.tile_pool(name="o_ps", bufs=4, space="PSUM") as o_ps_pool,
            tc.tile_pool(name="op", bufs=2) as out_pool,
        ):
            atg_sb = atg_pool.tile([128, HT, FS], BF16, tag="atg")
            nc.sync.dma_start(
                out=atg_sb[:, :, :],
                in_=a2a_out[:, :].rearrange("a (p n) -> p a n", p=128),
            )
            for ft in range(FS // 128):
                o_ps = o_ps_pool.tile([128, 2, 512], F32, tag="o")
                for p in range(HT):
                    for j in range(2):
                        nc.tensor.matmul(
                            o_ps[:, j, :],
                            lhsT=atg_sb[:, p, 128 * ft : 128 * (ft + 1)],
                            rhs=wo_sb[:, p, 512 * j : 512 * (j + 1)],
                            start=(p == 0),
                            stop=(p == HT - 1),
                        )
                ot = out_pool.tile([128, HID], F32, tag="ot")
                nc.vector.tensor_copy(
                    out=ot[:, :].rearrange("p (j n) -> p j n", j=2),
                    in_=o_ps[:, :, :],
                )
                nc.sync.dma_start(
                    out=out[128 * ft : 128 * (ft + 1), :], in_=ot[:, :]
                )

    nc.compile()
    return nc


def _get_nc():
    if "nc" not in _CACHE:
        _CACHE["nc"] = _build()
    return _CACHE["nc"]


def _reference_fallback(query_input, source_input, bias, wq, wk, wv, wo):
    """Numpy fallback, only used if bias is unexpectedly nonzero."""
    q = np.einsum("bfh,hnd->bfnd", query_input, wq) * (DH**-0.5)
    k = np.einsum("bth,hnd->btnd", source_input, wk)
    v = np.einsum("bth,hnd->btnd", source_input, wv)
    logits = np.einsum("btnd,bfnd->bnft", k, q) + bias
    logits -= logits.max(axis=-1, keepdims=True)
    w = np.exp(logits)
    w /= w.sum(axis=-1, keepdims=True)
    attn = np.einsum("bnft,btnd->bfnd", w, v)
    return np.einsum("bfnd,ndh->bfh", attn, wo).astype(np.float32)


def make_in_maps(query_input, source_input, wq, wk, wv, wo):
    wo2 = np.ascontiguousarray(wo.reshape(HID, HID).astype(NPBF16))
    qTb = np.ascontiguousarray(
        np.transpose(query_input, (0, 2, 1))
    ).astype(NPBF16)  # [B, HID, F]
    sTb = np.ascontiguousarray(np.transpose(source_input, (0, 2, 1))).astype(NPBF16)
    wqh = wq.reshape(HID, NH, DH)
    wkh = wk.reshape(HID, NH, DH)
    wvh = wv.reshape(HID, NH, DH)

    in_maps = []
    for c in range(8):
        sl = np.s_[:, 2 * c : 2 * c + 2, :]
        w3c = np.concatenate(
            [
                wqh[sl].reshape(HID, 128),
                wkh[sl].reshape(HID, 128),
                wvh[sl].reshape(HID, 128),
            ],
            axis=1,
        )
        in_maps.append(
            {
                "qT": qTb,
                "sT": sTb,
                "w3": np.ascontiguousarray(w3c).astype(NPBF16),
                "wo": wo2,
            }
        )
    return in_maps


def kernel(query_input, source_input, bias, wq, wk, wv, wo):
    query_input = np.asarray(query_input, dtype=np.float32)
    source_input = np.asarray(source_input, dtype=np.float32)
    bias = np.asarray(bias, dtype=np.float32)
    wq = np.asarray(wq, dtype=np.float32)
    wk = np.asarray(wk, dtype=np.float32)
    wv = np.asarray(wv, dtype=np.float32)
    wo = np.asarray(wo, dtype=np.float32)

    if np.any(bias):
        return _reference_fallback(query_input, source_input, bias, wq, wk, wv, wo)

    in_maps = make_in_maps(query_input, source_input, wq, wk, wv, wo)
    nc = _get_nc()
    res = run_bass_kernel_spmd(nc, in_maps, core_ids=list(range(8)))

    out_full = np.empty((B, F, HID), dtype=np.float32)
    for c in range(8):
        b, r = c // 4, c % 4
        out_full[b, FS * r : FS * (r + 1), :] = res.results[c]["out"]
    return out_full


# revision 33
# speedup vs baseline: 1.1944x; 1.0242x over previous
"""Multi-head attention (B=2, F=T=2048, H=1024, 16 heads x 64) on 8 TRN2
NeuronCores.

Sharding: pure head/tensor parallelism with an output-side AllToAll.
Core c owns heads {2c, 2c+1} for BOTH batches. Each core:
  1. projects Q^T / K^T / V for its 2 heads over the full sequences
     (both batches, front-loaded so attention owns all 8 PSUM banks),
  2. runs attention for its heads, software-pipelined so the ACT engine
     (exp) is the critical path: S/exp for step i+1 are emitted before
     the P@V matmuls of step i. Softmax denominators come free from a
     ones-column appended to V in the P@V matmul; exp folds the
     1/sqrt(64) logit scale into its free affine,
  3. normalizes A^T with a scale chain (DVE reciprocal -> Kc=1 PE
     ones-broadcast -> DVE multiply) deferred into the next f-chunk's
     loop so the in-order PE never stalls on it; each finished (batch,
     f-chunk) shard is DMA'd straight into the AllToAll input buffer,
  4. one 8-core AllToAll redistributes A^T from head-sharded to
     (batch, query-slice)-sharded, and the output projection runs locally
     with the full 1024-deep head contraction -> exact [512, 1024] slice.
Host concatenates the 8 slices. All matmuls run in bf16 with fp32 PSUM
accumulation.
"""

from contextlib import ExitStack

import ml_dtypes
import numpy as np

import concourse.bass as bass  # noqa: F401
import concourse.mybir as mybir
import concourse.tile as tile
from concourse import bacc
from concourse.bass_utils import run_bass_kernel_spmd

B, F, T, HID, NH, DH = 2, 2048, 2048, 1024, 16, 64
FS = F // 4  # 512-row output slice per core
HT = HID // 128  # 8 h-tiles
TT = T // 128  # 16 key tiles
FC = F // 512  # 4 query chunks
BF16, F32 = mybir.dt.bfloat16, mybir.dt.float32
NPBF16 = ml_dtypes.bfloat16

_CACHE: dict = {}


def _build():
    nc = bacc.Bacc("TRN2", target_bir_lowering=False, debug=False, num_devices=8)

    qT = nc.declare_dram_parameter("qT", [B, HID, F], BF16, isOutput=False)
    sT = nc.declare_dram_parameter("sT", [B, HID, T], BF16, isOutput=False)
    w3 = nc.declare_dram_parameter("w3", [HID, 384], BF16, isOutput=False)
    wo = nc.declare_dram_parameter("wo", [HID, HID], BF16, isOutput=False)
    out = nc.declare_dram_parameter("out", [FS, HID], F32, isOutput=True)

    seg = 128 * FS  # one A^T shard: [128 hd, 512 f]
    a2a_in = nc.dram_tensor("a2a_in", [8, seg], BF16)
    a2a_out = nc.dram_tensor("a2a_out", [8, seg], BF16)

    with tile.TileContext(nc) as tc, ExitStack() as ctx:
        persist = ctx.enter_context(tc.tile_pool(name="persist", bufs=1))
        kT_sb = persist.tile([128, B, T], BF16, tag="kT")
        v_sb = persist.tile([128, B, TT, 2, DH + 1], BF16, tag="v")
        qTp_sb = persist.tile([128, B, F], BF16, tag="qTp")
        wo_sb = persist.tile([128, HT, HID], BF16, tag="wo")
        w3_sb = persist.tile([128, HT, 3, 128], BF16, tag="w3")  # wq|wk|wv
        ones_sb = persist.tile([128, DH, 1], BF16, tag="ones")

        nc.vector.memset(ones_sb[:, :, :], 1.0)
        nc.vector.memset(v_sb[:, :, :, :, DH : DH + 1], 1.0)
        nc.sync.dma_start(
            out=w3_sb[:, :, :, :],
            in_=w3[:, :].rearrange("(a p) (k n) -> p a k n", p=128, n=128),
        )

        with (
            tc.tile_pool(name="inp", bufs=1) as inp_pool,
            tc.tile_pool(name="inps", bufs=2) as inps_pool,
            tc.tile_pool(name="ptp", bufs=4) as pt_pool,
            tc.tile_pool(name="rtp", bufs=2) as rt_pool,
            tc.tile_pool(name="stg", bufs=2) as stg_pool,
        ):
            # ---- projections (both batches, PSUM scope closes after) --
            with tc.tile_pool(name="proj_ps", bufs=2, space="PSUM") as proj_ps:
                for b in range(B):
                    sT_sb = inps_pool.tile([128, HT, T], BF16, tag="sT")
                    nc.sync.dma_start(
                        out=sT_sb[:, :, :],
                        in_=sT[b, :, :].rearrange("(a p) n -> p a n", p=128),
                    )
                    # K^T [128 hd, T]
                    for c in range(T // 512):
                        ps = proj_ps.tile([128, 512], F32, tag="ps")
                        for ht in range(HT):
                            nc.tensor.matmul(
                                ps[:, :],
                                lhsT=w3_sb[:, ht, 1, :],
                                rhs=sT_sb[:, ht, 512 * c : 512 * (c + 1)],
                                start=(ht == 0),
                                stop=(ht == HT - 1),
                            )
                        nc.vector.tensor_copy(
                            out=kT_sb[:, b, 512 * c : 512 * (c + 1)], in_=ps[:, :]
                        )
                    # V [t, 2*DH] per key tile
                    for tt in range(TT):
                        ps = proj_ps.tile([128, 128], F32, tag="ps")
                        for ht in range(HT):
                            nc.tensor.matmul(
                                ps[:, :],
                                lhsT=sT_sb[:, ht, 128 * tt : 128 * (tt + 1)],
                                rhs=w3_sb[:, ht, 2, :],
                                start=(ht == 0),
                                stop=(ht == HT - 1),
                            )
                        nc.vector.tensor_copy(
                            out=v_sb[:, b, tt, :, 0:DH],
                            in_=ps[:, :].rearrange("p (j d) -> p j d", j=2),
                        )
                    # Q^T [128 hd, F]
                    qT_sb = inp_pool.tile([128, HT, F], BF16, tag="qT")
                    nc.sync.dma_start(
                        out=qT_sb[:, :, :],
                        in_=qT[b, :, :].rearrange("(a p) n -> p a n", p=128),
                    )
                    for c in range(FC):
                        ps = proj_ps.tile([128, 512], F32, tag="ps")
                        for ht in range(HT):
                            nc.tensor.matmul(
                                ps[:, :],
                                lhsT=w3_sb[:, ht, 0, :],
                                rhs=qT_sb[:, ht, 512 * c : 512 * (c + 1)],
                                start=(ht == 0),
                                stop=(ht == HT - 1),
                            )
                        nc.vector.tensor_copy(
                            out=qTp_sb[:, b, 512 * c : 512 * (c + 1)], in_=ps[:, :]
                        )

            # ---- attention (both batches); scale chains deferred by one
            # fc so the in-order PE never stalls on recip/broadcast ------
            with (
                tc.tile_pool(name="s_ps", bufs=2, space="PSUM") as s_ps_pool,
                tc.tile_pool(name="a_ps", bufs=2, space="PSUM") as a_ps_pool,
            ):

                def flush_scale(b, fc, a_ps):
                    shard = a2a_in[4 * b + fc, :].rearrange(
                        "(p n) -> p n", p=128
                    )
                    rts = []
                    for j in range(2):
                        rt = rt_pool.tile([65, 1, 512], BF16, tag="rt")
                        with nc.allow_low_precision("bf16 softmax denom recip"):
                            nc.vector.reciprocal(
                                out=rt[64:65, 0, :], in_=a_ps[64:65, j, :]
                            )
                        rts.append(rt)
                    for j in range(2):
                        rt = rts[j]
                        bc = s_ps_pool.tile([64, 512], F32, tag="s")
                        nc.tensor.matmul(
                            bc[:, :],
                            lhsT=ones_sb[64:65, :, 0],
                            rhs=rt[64:65, 0, :],
                            start=True,
                            stop=True,
                        )
                        bc_sb = rt_pool.tile([64, 512], F32, tag="bc")
                        nc.vector.tensor_copy(out=bc_sb[:, :], in_=bc[:, :])
                        st = stg_pool.tile([64, 512], BF16, tag="st")
                        nc.vector.tensor_mul(
                            out=st[:, :], in0=a_ps[0:64, j, :], in1=bc_sb[:, :]
                        )
                        nc.sync.dma_start(
                            out=shard[64 * j : 64 * (j + 1), :], in_=st[:, :]
                        )

                def emit_s_exp(b, fc, tt):
                    sp = s_ps_pool.tile([128, 2, 512], F32, tag="s")
                    for j in range(2):
                        nc.tensor.matmul(
                            sp[:, j, :],
                            lhsT=kT_sb[
                                64 * j : 64 * (j + 1), b, 128 * tt : 128 * (tt + 1)
                            ],
                            rhs=qTp_sb[
                                64 * j : 64 * (j + 1), b, 512 * fc : 512 * (fc + 1)
                            ],
                            start=True,
                            stop=True,
                        )
                    pt = pt_pool.tile([128, 2, 512], BF16, tag="pt")
                    nc.scalar.activation(
                        out=pt[:, :, :],
                        in_=sp[:, :, :],
                        func=mybir.ActivationFunctionType.Exp,
                        scale=float(DH) ** -0.5,
                    )
                    return pt

                # software-pipelined: S/exp run one (b,fc,tt) step ahead of
                # the P@V accumulation so the in-order PE never waits on exp
                steps = [
                    (b, fc, tt) for b in range(B) for fc in range(FC)
                    for tt in range(TT)
                ]
                pending = None
                a_tiles = {}
                pts = {}
                pts[steps[0]] = emit_s_exp(*steps[0])
                for i, (b, fc, tt) in enumerate(steps):
                    if tt == 0:
                        a_tiles[(b, fc)] = a_ps_pool.tile(
                            [65, 2, 512], F32, tag="a", name="a_acc"
                        )
                    if i + 1 < len(steps):
                        pts[steps[i + 1]] = emit_s_exp(*steps[i + 1])
                    a_ps = a_tiles[(b, fc)]
                    pt = pts.pop((b, fc, tt))
                    for j in range(2):
                        nc.tensor.matmul(
                            a_ps[:, j, :],
                            lhsT=v_sb[:, b, tt, j, :],
                            rhs=pt[:, j, :],
                            start=(tt == 0),
                            stop=(tt == TT - 1),
                        )
                    if tt == 10 and pending is not None:
                        flush_scale(*pending)
                        pending = None
                    if tt == TT - 1:
                        pending = (b, fc, a_ps)
                flush_scale(*pending)

        nc.scalar.dma_start(
            out=wo_sb[:, :, :], in_=wo[:, :].rearrange("(a p) n -> p a n", p=128)
        )

        # ---- AllToAll: head-sharded -> (batch, f-slice)-sharded -------
        nc.gpsimd.collective_compute(
            "AllToAll",
            mybir.AluOpType.bypass,
            replica_groups=[[0, 1, 2, 3, 4, 5, 6, 7]],
            ins=[a2a_in.ap().opt()],
            outs=[a2a_out.ap().opt()],
        )

        with (
            tc.tile_pool(name="atg", bufs=1) as atg_pool,
            tc.tile_pool(name="o_ps", bufs=4, space="PSUM") as o_ps_pool,
            tc.tile_pool(name="op", bufs=2) as out_pool,
        ):
            atg_sb = atg_pool.tile([128, HT, FS], BF16, tag="atg")
            nc.sync.dma_start(
                out=atg_sb[:, :, :],
                in_=a2a_out[:, :].rearrange("a (p n) -> p a n", p=128),
            )
            for ft in range(FS // 128):
                o_ps = o_ps_pool.tile([128, 2, 512], F32, tag="o")
                for p in range(HT):
                    for j in range(2):
                        nc.tensor.matmul(
                            o_ps[:, j, :],
                            lhsT=atg_sb[:, p, 128 * ft : 128 * (ft + 1)],
                            rhs=wo_sb[:, p, 512 * j : 512 * (j + 1)],
                            start=(p == 0),
                            stop=(p == HT - 1),
                        )
                ot = out_pool.tile([128, HID], F32, tag="ot")
                nc.vector.tensor_copy(
                    out=ot[:, :].rearrange("p (j n) -> p j n", j=2),
                    in_=o_ps[:, :, :],
                )
                nc.sync.dma_start(
                    out=out[128 * ft : 128 * (ft + 1), :], in_=ot[:, :]
                )

    nc.compile()
    return nc


def _get_nc():
    if "nc" not in _CACHE:
        _CACHE["nc"] = _build()
    return _CACHE["nc"]


def _reference_fallback(query_input, source_input, bias, wq, wk, wv, wo):
    """Numpy fallback, only used if bias is unexpectedly nonzero."""
    q = np.einsum("bfh,hnd->bfnd", query_input, wq) * (DH**-0.5)
    k = np.einsum("bth,hnd->btnd", source_input, wk)
    v = np.einsum("bth,hnd->btnd", source_input, wv)
    logits = np.einsum("btnd,bfnd->bnft", k, q) + bias
    logits -= logits.max(axis=-1, keepdims=True)
    w = np.exp(logits)
    w /= w.sum(axis=-1, keepdims=True)
    attn = np.einsum("bnft,btnd->bfnd", w, v)
    return np.einsum("bfnd,ndh->bfh", attn, wo).astype(np.float32)


def make_in_maps(query_input, source_input, wq, wk, wv, wo):
    wo2 = np.ascontiguousarray(wo.reshape(HID, HID).astype(NPBF16))
    qTb = np.ascontiguousarray(
        np.transpose(query_input, (0, 2, 1))
    ).astype(NPBF16)  # [B, HID, F]
    sTb = np.ascontiguousarray(np.transpose(source_input, (0, 2, 1))).astype(NPBF16)
    wqh = wq.reshape(HID, NH, DH)
    wkh = wk.reshape(HID, NH, DH)
    wvh = wv.reshape(HID, NH, DH)

    in_maps = []
    for c in range(8):
        sl = np.s_[:, 2 * c : 2 * c + 2, :]
        w3c = np.concatenate(
            [
                wqh[sl].reshape(HID, 128),
                wkh[sl].reshape(HID, 128),
                wvh[sl].reshape(HID, 128),
            ],
            axis=1,
        )
        in_maps.append(
            {
                "qT": qTb,
                "sT": sTb,
                "w3": np.ascontiguousarray(w3c).astype(NPBF16),
                "wo": wo2,
            }
        )
    return in_maps


def kernel(query_input, source_input, bias, wq, wk, wv, wo):
    query_input = np.asarray(query_input, dtype=np.float32)
    source_input = np.asarray(source_input, dtype=np.float32)
    bias = np.asarray(bias, dtype=np.float32)
    wq = np.asarray(wq, dtype=np.float32)
    wk = np.asarray(wk, dtype=np.float32)
    wv = np.asarray(wv, dtype=np.float32)
    wo = np.asarray(wo, dtype=np.float32)

    if np.any(bias):
        return _reference_fallback(query_input, source_input, bias, wq, wk, wv, wo)

    in_maps = make_in_maps(query_input, source_input, wq, wk, wv, wo)
    nc = _get_nc()
    res = run_bass_kernel_spmd(nc, in_maps, core_ids=list(range(8)))

    out_full = np.empty((B, F, HID), dtype=np.float32)
    for c in range(8):
        b, r = c // 4, c % 4
        out_full[b, FS * r : FS * (r + 1), :] = res.results[c]["out"]
    return out_full


# revision 34
# speedup vs baseline: 1.2020x; 1.0063x over previous
"""Multi-head attention (B=2, F=T=2048, H=1024, 16 heads x 64) on 8 TRN2
NeuronCores.

Sharding: pure head/tensor parallelism with an output-side AllToAll.
Core c owns heads {2c, 2c+1} for BOTH batches. Each core:
  1. projects Q^T / K^T / V for its 2 heads over the full sequences
     (both batches, front-loaded so attention owns all 8 PSUM banks),
  2. runs attention for its heads, software-pipelined so the ACT engine
     (exp) is the critical path: S/exp for step i+1 are emitted before
     the P@V matmuls of step i. Softmax denominators come free from a
     ones-column appended to V in the P@V matmul; exp folds the
     1/sqrt(64) logit scale into its free affine,
  3. normalizes A^T with a scale chain (DVE reciprocal -> Kc=1 PE
     ones-broadcast -> DVE multiply) deferred into the next f-chunk's
     loop so the in-order PE never stalls on it; each finished (batch,
     f-chunk) shard is DMA'd straight into the AllToAll input buffer,
  4. one 8-core AllToAll redistributes A^T from head-sharded to
     (batch, query-slice)-sharded, and the output projection runs locally
     with the full 1024-deep head contraction -> exact [512, 1024] slice.
Host concatenates the 8 slices. All matmuls run in bf16 with fp32 PSUM
accumulation.
"""

from contextlib import ExitStack

import ml_dtypes
import numpy as np

import concourse.bass as bass  # noqa: F401
import concourse.mybir as mybir
import concourse.tile as tile
from concourse import bacc
from concourse.bass_utils import run_bass_kernel_spmd

B, F, T, HID, NH, DH = 2, 2048, 2048, 1024, 16, 64
FS = F // 4  # 512-row output slice per core
HT = HID // 128  # 8 h-tiles
TT = T // 128  # 16 key tiles
FC = F // 512  # 4 query chunks
BF16, F32 = mybir.dt.bfloat16, mybir.dt.float32
NPBF16 = ml_dtypes.bfloat16

_CACHE: dict = {}


def _build():
    nc = bacc.Bacc("TRN2", target_bir_lowering=False, debug=False, num_devices=8)

    qT = nc.declare_dram_parameter("qT", [B, HID, F], BF16, isOutput=False)
    sT = nc.declare_dram_parameter("sT", [B, HID, T], BF16, isOutput=False)
    w3 = nc.declare_dram_parameter("w3", [HID, 384], BF16, isOutput=False)
    wo = nc.declare_dram_parameter("wo", [HID, HID], BF16, isOutput=False)
    out = nc.declare_dram_parameter("out", [FS, HID], F32, isOutput=True)

    seg = 128 * FS  # one A^T shard: [128 hd, 512 f]
    a2a_in = nc.dram_tensor("a2a_in", [8, seg], BF16)
    a2a_out = nc.dram_tensor("a2a_out", [8, seg], BF16)

    with tile.TileContext(nc) as tc, ExitStack() as ctx:
        persist = ctx.enter_context(tc.tile_pool(name="persist", bufs=1))
        kT_sb = persist.tile([128, B, T], BF16, tag="kT")
        v_sb = persist.tile([128, B, TT, 2, DH + 1], BF16, tag="v")
        qTp_sb = persist.tile([128, B, F], BF16, tag="qTp")
        wo_sb = persist.tile([128, HT, HID], BF16, tag="wo")
        w3_sb = persist.tile([128, HT, 3, 128], BF16, tag="w3")  # wq|wk|wv
        ones_sb = persist.tile([128, DH, 1], BF16, tag="ones")

        nc.vector.memset(ones_sb[:, :, :], 1.0)
        nc.vector.memset(v_sb[:, :, :, :, DH : DH + 1], 1.0)
        nc.sync.dma_start(
            out=w3_sb[:, :, :, :],
            in_=w3[:, :].rearrange("(a p) (k n) -> p a k n", p=128, n=128),
        )

        with (
            tc.tile_pool(name="inp", bufs=1) as inp_pool,
            tc.tile_pool(name="inps", bufs=2) as inps_pool,
            tc.tile_pool(name="ptp", bufs=6) as pt_pool,
            tc.tile_pool(name="rtp", bufs=4) as rt_pool,
            tc.tile_pool(name="stg", bufs=4) as stg_pool,
        ):
            # ---- projections (both batches, PSUM scope closes after) --
            with tc.tile_pool(name="proj_ps", bufs=2, space="PSUM") as proj_ps:
                for b in range(B):
                    sT_sb = inps_pool.tile([128, HT, T], BF16, tag="sT")
                    nc.sync.dma_start(
                        out=sT_sb[:, :, :],
                        in_=sT[b, :, :].rearrange("(a p) n -> p a n", p=128),
                    )
                    # K^T [128 hd, T]
                    for c in range(T // 512):
                        ps = proj_ps.tile([128, 512], F32, tag="ps")
                        for ht in range(HT):
                            nc.tensor.matmul(
                                ps[:, :],
                                lhsT=w3_sb[:, ht, 1, :],
                                rhs=sT_sb[:, ht, 512 * c : 512 * (c + 1)],
                                start=(ht == 0),
                                stop=(ht == HT - 1),
                            )
                        nc.vector.tensor_copy(
                            out=kT_sb[:, b, 512 * c : 512 * (c + 1)], in_=ps[:, :]
                        )
                    # V [t, 2*DH] per key tile
                    for tt in range(TT):
                        ps = proj_ps.tile([128, 128], F32, tag="ps")
                        for ht in range(HT):
                            nc.tensor.matmul(
                                ps[:, :],
                                lhsT=sT_sb[:, ht, 128 * tt : 128 * (tt + 1)],
                                rhs=w3_sb[:, ht, 2, :],
                                start=(ht == 0),
                                stop=(ht == HT - 1),
                            )
                        nc.vector.tensor_copy(
                            out=v_sb[:, b, tt, :, 0:DH],
                            in_=ps[:, :].rearrange("p (j d) -> p j d", j=2),
                        )
                    # Q^T [128 hd, F]
                    qT_sb = inp_pool.tile([128, HT, F], BF16, tag="qT")
                    nc.sync.dma_start(
                        out=qT_sb[:, :, :],
                        in_=qT[b, :, :].rearrange("(a p) n -> p a n", p=128),
                    )
                    for c in range(FC):
                        ps = proj_ps.tile([128, 512], F32, tag="ps")
                        for ht in range(HT):
                            nc.tensor.matmul(
                                ps[:, :],
                                lhsT=w3_sb[:, ht, 0, :],
                                rhs=qT_sb[:, ht, 512 * c : 512 * (c + 1)],
                                start=(ht == 0),
                                stop=(ht == HT - 1),
                            )
                        nc.vector.tensor_copy(
                            out=qTp_sb[:, b, 512 * c : 512 * (c + 1)], in_=ps[:, :]
                        )

            # ---- attention (both batches); scale chains deferred by one
            # fc so the in-order PE never stalls on recip/broadcast ------
            with (
                tc.tile_pool(name="s_ps", bufs=2, space="PSUM") as s_ps_pool,
                tc.tile_pool(name="a_ps", bufs=2, space="PSUM") as a_ps_pool,
            ):

                def flush_scale(b, fc, a_ps):
                    shard = a2a_in[4 * b + fc, :].rearrange(
                        "(p n) -> p n", p=128
                    )
                    rts = []
                    for j in range(2):
                        rt = rt_pool.tile([65, 1, 512], BF16, tag="rt")
                        with nc.allow_low_precision("bf16 softmax denom recip"):
                            nc.vector.reciprocal(
                                out=rt[64:65, 0, :], in_=a_ps[64:65, j, :]
                            )
                        rts.append(rt)
                    for j in range(2):
                        rt = rts[j]
                        bc = s_ps_pool.tile([64, 512], F32, tag="s")
                        nc.tensor.matmul(
                            bc[:, :],
                            lhsT=ones_sb[64:65, :, 0],
                            rhs=rt[64:65, 0, :],
                            start=True,
                            stop=True,
                        )
                        bc_sb = rt_pool.tile([64, 512], F32, tag="bc")
                        nc.vector.tensor_copy(out=bc_sb[:, :], in_=bc[:, :])
                        st = stg_pool.tile([64, 512], BF16, tag="st")
                        nc.vector.tensor_mul(
                            out=st[:, :], in0=a_ps[0:64, j, :], in1=bc_sb[:, :]
                        )
                        nc.sync.dma_start(
                            out=shard[64 * j : 64 * (j + 1), :], in_=st[:, :]
                        )

                def emit_s_exp(b, fc, tt):
                    sp = s_ps_pool.tile([128, 2, 512], F32, tag="s")
                    for j in range(2):
                        nc.tensor.matmul(
                            sp[:, j, :],
                            lhsT=kT_sb[
                                64 * j : 64 * (j + 1), b, 128 * tt : 128 * (tt + 1)
                            ],
                            rhs=qTp_sb[
                                64 * j : 64 * (j + 1), b, 512 * fc : 512 * (fc + 1)
                            ],
                            start=True,
                            stop=True,
                        )
                    pt = pt_pool.tile([128, 2, 512], BF16, tag="pt")
                    nc.scalar.activation(
                        out=pt[:, :, :],
                        in_=sp[:, :, :],
                        func=mybir.ActivationFunctionType.Exp,
                        scale=float(DH) ** -0.5,
                    )
                    return pt

                # software-pipelined: S/exp run one (b,fc,tt) step ahead of
                # the P@V accumulation so the in-order PE never waits on exp
                steps = [
                    (b, fc, tt) for b in range(B) for fc in range(FC)
                    for tt in range(TT)
                ]
                pending = None
                a_tiles = {}
                pts = {}
                pts[steps[0]] = emit_s_exp(*steps[0])
                for i, (b, fc, tt) in enumerate(steps):
                    if tt == 0:
                        a_tiles[(b, fc)] = a_ps_pool.tile(
                            [65, 2, 512], F32, tag="a", name="a_acc"
                        )
                    if i + 1 < len(steps):
                        pts[steps[i + 1]] = emit_s_exp(*steps[i + 1])
                    a_ps = a_tiles[(b, fc)]
                    pt = pts.pop((b, fc, tt))
                    for j in range(2):
                        nc.tensor.matmul(
                            a_ps[:, j, :],
                            lhsT=v_sb[:, b, tt, j, :],
                            rhs=pt[:, j, :],
                            start=(tt == 0),
                            stop=(tt == TT - 1),
                        )
                    if tt == 10 and pending is not None:
                        flush_scale(*pending)
                        pending = None
                    if tt == TT - 1:
                        pending = (b, fc, a_ps)
                flush_scale(*pending)

        nc.scalar.dma_start(
            out=wo_sb[:, :, :], in_=wo[:, :].rearrange("(a p) n -> p a n", p=128)
        )

        # ---- AllToAll: head-sharded -> (batch, f-slice)-sharded -------
        nc.gpsimd.collective_compute(
            "AllToAll",
            mybir.AluOpType.bypass,
            replica_groups=[[0, 1, 2, 3, 4, 5, 6, 7]],
            ins=[a2a_in.ap().opt()],
            outs=[a2a_out.ap().opt()],
        )

        with (
            tc.tile_pool(name="atg", bufs=1) as atg_pool,
            tc.tile_pool(name="o_ps", bufs=4, space="PSUM") as o_ps_pool,
            tc.tile_pool(name="op", bufs=2) as out_pool,
        ):
            atg_sb = atg_pool.tile([128, HT, FS], BF16, tag="atg")
            nc.sync.dma_start(
                out=atg_sb[:, :, :],
                in_=a2a_out[:, :].rearrange("a (p n) -> p a n", p=128),
            )
            for ft in range(FS // 128):
                o_ps = o_ps_pool.tile([128, 2, 512], F32, tag="o")
                for p in range(HT):
                    for j in range(2):
                        nc.tensor.matmul(
                            o_ps[:, j, :],
                            lhsT=atg_sb[:, p, 128 * ft : 128 * (ft + 1)],
                            rhs=wo_sb[:, p, 512 * j : 512 * (j + 1)],
                            start=(p == 0),
                            stop=(p == HT - 1),
                        )
                ot = out_pool.tile([128, HID], F32, tag="ot")
                nc.vector.tensor_copy(
                    out=ot[:, :].rearrange("p (j n) -> p j n", j=2),
                    in_=o_ps[:, :, :],
                )
                nc.sync.dma_start(
                    out=out[128 * ft : 128 * (ft + 1), :], in_=ot[:, :]
                )

    nc.compile()
    return nc


def _get_nc():
    if "nc" not in _CACHE:
        _CACHE["nc"] = _build()
    return _CACHE["nc"]


def _reference_fallback(query_input, source_input, bias, wq, wk, wv, wo):
    """Numpy fallback, only used if bias is unexpectedly nonzero."""
    q = np.einsum("bfh,hnd->bfnd", query_input, wq) * (DH**-0.5)
    k = np.einsum("bth,hnd->btnd", source_input, wk)
    v = np.einsum("bth,hnd->btnd", source_input, wv)
    logits = np.einsum("btnd,bfnd->bnft", k, q) + bias
    logits -= logits.max(axis=-1, keepdims=True)
    w = np.exp(logits)
    w /= w.sum(axis=-1, keepdims=True)
    attn = np.einsum("bnft,btnd->bfnd", w, v)
    return np.einsum("bfnd,ndh->bfh", attn, wo).astype(np.float32)


def make_in_maps(query_input, source_input, wq, wk, wv, wo):
    wo2 = np.ascontiguousarray(wo.reshape(HID, HID).astype(NPBF16))
    qTb = np.ascontiguousarray(
        np.transpose(query_input, (0, 2, 1))
    ).astype(NPBF16)  # [B, HID, F]
    sTb = np.ascontiguousarray(np.transpose(source_input, (0, 2, 1))).astype(NPBF16)
    wqh = wq.reshape(HID, NH, DH)
    wkh = wk.reshape(HID, NH, DH)
    wvh = wv.reshape(HID, NH, DH)

    in_maps = []
    for c in range(8):
        sl = np.s_[:, 2 * c : 2 * c + 2, :]
        w3c = np.concatenate(
            [
                wqh[sl].reshape(HID, 128),
                wkh[sl].reshape(HID, 128),
                wvh[sl].reshape(HID, 128),
            ],
            axis=1,
        )
        in_maps.append(
            {
                "qT": qTb,
                "sT": sTb,
                "w3": np.ascontiguousarray(w3c).astype(NPBF16),
                "wo": wo2,
            }
        )
    return in_maps


def kernel(query_input, source_input, bias, wq, wk, wv, wo):
    query_input = np.asarray(query_input, dtype=np.float32)
    source_input = np.asarray(source_input, dtype=np.float32)
    bias = np.asarray(bias, dtype=np.float32)
    wq = np.asarray(wq, dtype=np.float32)
    wk = np.asarray(wk, dtype=np.float32)
    wv = np.asarray(wv, dtype=np.float32)
    wo = np.asarray(wo, dtype=np.float32)

    if np.any(bias):
        return _reference_fallback(query_input, source_input, bias, wq, wk, wv, wo)

    in_maps = make_in_maps(query_input, source_input, wq, wk, wv, wo)
    nc = _get_nc()
    res = run_bass_kernel_spmd(nc, in_maps, core_ids=list(range(8)))

    out_full = np.empty((B, F, HID), dtype=np.float32)
    for c in range(8):
        b, r = c // 4, c % 4
        out_full[b, FS * r : FS * (r + 1), :] = res.results[c]["out"]
    return out_full
